# revision 6
# baseline (speedup 1.0000x reference)
"""Fused attention block (QKV proj -> softmax attention -> out proj -> residual+LN)
for B=4, S=2048, D=512, H=8, DH=64 on 8 TRN2 NeuronCores.

v3 (from v2 HW trace): fp8 DoubleRow streams at 1 out-row/cycle on HW (not the
cost model's 0.5), so DR only pays when it deepens the contraction: keep fp8-DR
for the 512-deep QKV/out projections (2x fewer passes) and the 2048-deep ctx
accumulation (2x128-key k-tiles), but run scores as plain bf16 matmuls (K=64 -
DR can't help), which also removes the Q/K fp8 quantization error. The PE was
stuck at 1.2 GHz (70% busy: in-order PE stream blocked at every ctx matmul
waiting for exp): v3 software-pipelines the PE stream - ctx matmuls for
key-pair kp are emitted after the scores of kp+1, so exp has a full kc-pair of
PE work to hide behind; cx accumulators double-buffer so head h+1 never waits
on head h's normalize chain.

Probs are fp8e5m2 (range to 57k: no max-subtraction at raw-score|max| ~77).
Exp splits scalar (true Exp) / DVE (Schraudolph: affine fp32->uint8 convert
whose bits read as e5m2) 11:5 per 16 kc. Softmax denominators ride row 64 of
ctx^T via a ones-column in V (padded to 80-col head stride: dual-fp8 ldweights
needs 16B-aligned outer steps). Residual-add is folded into the out-projection
as an identity-weights bf16 matmul; rstd = exp(-0.5*ln(var+eps)) keeps the
scalar engine on one activation table all kernel. bv folds into the residual on
host (ctx_norm@Wo + (bo + Wo@bv) + x).

Sharding: token-parallel, zero collectives. Core c handles batch b=c//2, query
tokens [(c%2)*1024, (c%2+1)*1024); K/V for the full sequence are computed
redundantly per core.
"""

import os
import sys

import numpy as np

for _p in ("/opt/trn_rl_repo",):
    if os.path.isdir(_p) and _p not in sys.path:
        sys.path.insert(0, _p)

import ml_dtypes

import concourse.bacc as bacc
import concourse.tile as tile
from concourse import mybir
from concourse.bass_utils import run_bass_kernel_spmd

BF16 = mybir.dt.bfloat16
F32 = mybir.dt.float32
E4 = mybir.dt.float8e4
E5 = mybir.dt.float8e5
U8 = mybir.dt.uint8
AF = mybir.ActivationFunctionType
ALU = mybir.AluOpType
DR = mybir.MatmulPerfMode.DoubleRow

P = 128
D = 512
DH = 64
H = 8
S = 2048
TQ = 1024
B = 4
NCORES = 8
EPS = 1e-5

# Schraudolph e5m2 exp on raw scores: bits = st*SCHRAU_A + SCHRAU_B (uint8).
SCHRAU_A = 0.125 * (4.0 / np.log(2.0))
SCHRAU_B = 60.2

# Per-kc exp engine: 'S' scalar true exp, 'D' DVE Schraudolph. 11 S / 5 D.
EXP_PATTERN = "SSDSSDSSDSSDSSDS"

TRACE = False
LAST_RESULTS = None
_NC_CACHE = None


def _build():
    nc = bacc.Bacc()

    xt8d = nc.declare_dram_parameter("xt8", [P, 2, 2, S], E4, isOutput=False)
    xtq8d = nc.declare_dram_parameter("xtq8", [P, 2, 2, TQ], E4, isOutput=False)
    wq8d = nc.declare_dram_parameter("wq8", [P, 2, 2, D], E4, isOutput=False)
    wk8d = nc.declare_dram_parameter("wk8", [P, 2, 2, D], E4, isOutput=False)
    wv8d = nc.declare_dram_parameter("wv8", [P, 2, 2, D], E4, isOutput=False)
    wo8d = nc.declare_dram_parameter("wo8", [P, 2, 2, D], E4, isOutput=False)
    bqd = nc.declare_dram_parameter("bqp", [P, 4], F32, isOutput=False)
    bkd = nc.declare_dram_parameter("bkp", [P, 4], F32, isOutput=False)
    xresd = nc.declare_dram_parameter("xres", [P, 8, D], BF16, isOutput=False)
    idd = nc.declare_dram_parameter("id128", [P, P], BF16, isOutput=False)
    gmd = nc.declare_dram_parameter("gamma", [P, D], F32, isOutput=False)
    btd = nc.declare_dram_parameter("beta", [P, D], F32, isOutput=False)
    outd = nc.declare_dram_parameter("out", [TQ, D], F32, isOutput=True)

    with tile.TileContext(nc) as tc:
        with (
            tc.tile_pool(name="big", bufs=1) as big,
            tc.tile_pool(name="work", bufs=3) as work,
            tc.tile_pool(name="ps_st", bufs=2, space="PSUM") as ps_st,
            tc.tile_pool(name="ps_cx", bufs=2, space="PSUM") as ps_cx,
        ):
            # ---------------- loads ----------------
            wk_sb = big.tile([P, 2, 2, D], E4)
            wq_sb = big.tile([P, 2, 2, D], E4)
            wv_sb = big.tile([P, 2, 2, D], E4)
            wo_sb = big.tile([P, 2, 2, D], E4)
            xt_sb = big.tile([P, 2, 2, S], E4)
            xtq_sb = big.tile([P, 2, 2, TQ], E4)
            nc.sync.dma_start(out=wk_sb[:, :, :, :], in_=wk8d[:, :, :, :])
            nc.sync.dma_start(out=xt_sb[:, :, :, :], in_=xt8d[:, :, :, :])
            nc.sync.dma_start(out=wq_sb[:, :, :, :], in_=wq8d[:, :, :, :])
            nc.sync.dma_start(out=xtq_sb[:, :, :, :], in_=xtq8d[:, :, :, :])
            nc.sync.dma_start(out=wv_sb[:, :, :, :], in_=wv8d[:, :, :, :])
            bq_sb = big.tile([P, 4], F32)
            bk_sb = big.tile([P, 4], F32)
            nc.sync.dma_start(out=bq_sb[:, :], in_=bqd[:, :])
            nc.sync.dma_start(out=bk_sb[:, :], in_=bkd[:, :])
            nc.sync.dma_start(out=wo_sb[:, :, :, :], in_=wo8d[:, :, :, :])
            xres_sb = big.tile([P, 8, D], BF16)
            id_sb = big.tile([P, P], BF16)
            gm_sb = big.tile([P, D], F32)
            bt_sb = big.tile([P, D], F32)
            nc.sync.dma_start(out=xres_sb[:, :, :], in_=xresd[:, :, :])
            nc.sync.dma_start(out=id_sb[:, :], in_=idd[:, :])
            nc.sync.dma_start(out=gm_sb[:, :], in_=gmd[:, :])
            nc.sync.dma_start(out=bt_sb[:, :], in_=btd[:, :])
            eps_sb = big.tile([P, 1], F32)
            nc.gpsimd.memset(eps_sb[:, :], EPS)

            qt_bf = big.tile([P, 4, TQ], BF16)
            kt_bf = big.tile([P, 4, S], BF16)
            vaug = big.tile([P, 16, H, 80], E5)  # 80-col head stride: 16B align
            nc.gpsimd.memset(vaug[:, :, :, 64:65], 1.0)
            pr8 = big.tile([P, 16, TQ], E5)
            ctxT8 = big.tile([P, 2, 2, TQ], E4)
            y_sb = big.tile([P, 8, D], F32)
            mv_all = big.tile([P, 8, 2], F32)
            lnu = big.tile([P, 8], F32)
            rstd_all = big.tile([P, 8], F32)

            copy_rr = [0]  # round-robin engine for PSUM->SBUF proj copies

            def proj_copy(dst, src, bias_ap):
                if copy_rr[0] % 2 == 0:
                    if bias_ap is None:
                        nc.scalar.activation(out=dst, in_=src, func=AF.Copy)
                    else:
                        nc.scalar.activation(
                            out=dst, in_=src, func=AF.Identity, bias=bias_ap
                        )
                else:
                    nc.vector.tensor_scalar(
                        dst, src, bias_ap if bias_ap is not None else 0.0,
                        None, ALU.add,
                    )
                copy_rr[0] += 1

            # ---------------- K/Q/V projections (fp8 DoubleRow) ----------------
            for mb in range(4):
                for j2 in range(2):
                    ps = ps_st.tile([P, 1024], F32, tag="st")
                    for half in range(2):
                        t4 = 2 * j2 + half
                        for a in range(2):
                            nc.tensor.matmul(
                                ps[:, half * 512:(half + 1) * 512],
                                lhsT=wk_sb[:, a, :, mb * P:(mb + 1) * P],
                                rhs=xt_sb[:, a, :, t4 * 512:(t4 + 1) * 512],
                                start=(a == 0),
                                stop=(a == 1),
                                perf_mode=DR,
                            )
                    proj_copy(
                        kt_bf[:, mb, 2 * j2 * 512:(2 * j2 + 2) * 512],
                        ps[:, :],
                        bk_sb[:, mb:mb + 1],
                    )
            for mb in range(4):
                ps = ps_st.tile([P, 1024], F32, tag="st")
                for half in range(2):
                    for a in range(2):
                        nc.tensor.matmul(
                            ps[:, half * 512:(half + 1) * 512],
                            lhsT=wq_sb[:, a, :, mb * P:(mb + 1) * P],
                            rhs=xtq_sb[:, a, :, half * 512:(half + 1) * 512],
                            start=(a == 0),
                            stop=(a == 1),
                            perf_mode=DR,
                        )
                proj_copy(qt_bf[:, mb, :], ps[:, :], bq_sb[:, mb:mb + 1])
            for j in range(8):
                ps = ps_st.tile([P, 1024], F32, tag="st")
                for half in range(2):
                    t16 = 2 * j + half
                    for a in range(2):
                        nc.tensor.matmul(
                            ps[:, half * 512:(half + 1) * 512],
                            lhsT=xt_sb[:, a, :, t16 * P:(t16 + 1) * P],
                            rhs=wv_sb[:, a, :, :],
                            start=(a == 0),
                            stop=(a == 1),
                            perf_mode=DR,
                        )
                proj_copy(
                    vaug[:, 2 * j:2 * j + 2, :, 0:64],
                    ps[:, :].rearrange("p (two h e) -> p two h e", two=2, h=H),
                    None,
                )

            # ---------------- attention (ctx lags scores by one kc-pair) ------
            pr8u = pr8.bitcast(U8)

            def scores_kc(h, kc, st):
                po = (h % 2) * 64
                chn = h // 2
                for half in range(2):
                    nc.tensor.matmul(
                        st[:, half * 512:(half + 1) * 512],
                        lhsT=kt_bf[po:po + 64, chn, kc * P:(kc + 1) * P],
                        rhs=qt_bf[po:po + 64, chn,
                                  half * 512:(half + 1) * 512],
                        start=True,
                        stop=True,
                    )
                if EXP_PATTERN[kc] == "S":
                    nc.scalar.activation(
                        out=pr8[:, kc, :], in_=st[:, :],
                        func=AF.Exp, scale=0.125,
                    )
                else:
                    nc.vector.tensor_scalar(
                        pr8u[:, kc, :], st[:, :],
                        float(SCHRAU_A), float(SCHRAU_B),
                        ALU.mult, ALU.add,
                    )

            def ctx_kp(h, kp, cx):
                for half in range(2):
                    nc.tensor.matmul(
                        cx[:, half * 512:(half + 1) * 512],
                        lhsT=vaug[:, 2 * kp:2 * kp + 2, h, 0:65],
                        rhs=pr8[:, 2 * kp:2 * kp + 2,
                                half * 512:(half + 1) * 512],
                        start=(kp == 0),
                        stop=(kp == 7),
                        perf_mode=DR,
                    )

            def normalize(h, cx):
                rec = work.tile([1, 1024], F32, tag="rec")
                nc.vector.reciprocal(rec[:, :], cx[64:65, :])
                recb = work.tile([64, 1024], F32, tag="recb")
                nc.gpsimd.partition_broadcast(recb[:, :], rec[:, :])
                nc.vector.tensor_tensor(
                    out=ctxT8[(h % 2) * 64:(h % 2) * 64 + 64,
                              h // 4, (h % 4) // 2, :],
                    in0=cx[0:64, :],
                    in1=recb[:, :],
                    op=ALU.mult,
                )

            # pending: (kind, args) queue so ctx trails scores by one kc-pair
            cx_tiles = {}
            pending = []  # list of ("ctx", h, kp) or ("norm", h)
            for h in range(H):
                cx_tiles[h] = ps_cx.tile([65, 1024], F32, tag="cx",
                                         name=f"cx{h}")
                for kp in range(8):
                    for j in range(2):
                        st = ps_st.tile([P, 1024], F32, tag="st")
                        scores_kc(h, 2 * kp + j, st)
                    while pending:
                        kind, *args = pending.pop(0)
                        if kind == "ctx":
                            hh, kpp = args
                            ctx_kp(hh, kpp, cx_tiles[hh])
                        else:
                            (hh,) = args
                            normalize(hh, cx_tiles[hh])
                            del cx_tiles[hh]
                    pending.append(("ctx", h, kp))
                    if kp == 7:
                        pending.append(("norm", h))
            while pending:
                kind, *args = pending.pop(0)
                if kind == "ctx":
                    hh, kpp = args
                    ctx_kp(hh, kpp, cx_tiles[hh])
                else:
                    (hh,) = args
                    normalize(hh, cx_tiles[hh])
                    del cx_tiles[hh]

            # ---------------- out proj + residual + LN ----------------
            for t8 in range(8):
                ps = ps_st.tile([P, 1024], F32, tag="st")
                for a in range(2):
                    nc.tensor.matmul(
                        ps[:, 0:512],
                        lhsT=ctxT8[:, a, :, t8 * P:(t8 + 1) * P],
                        rhs=wo_sb[:, a, :, :],
                        start=(a == 0),
                        stop=False,
                        perf_mode=DR,
                        skip_group_check=True,
                    )
                nc.tensor.matmul(
                    ps[:, 0:512],
                    lhsT=id_sb[:, :],
                    rhs=xres_sb[:, t8, :],
                    start=False,
                    stop=True,
                    skip_group_check=True,
                )
                stt6 = work.tile([P, 6], F32, tag="bn")
                nc.vector.bn_stats(out=stt6[:, :], in_=ps[:, 0:512])
                nc.vector.bn_aggr(out=mv_all[:, t8, :], in_=stt6[:, :])
                nc.scalar.activation(
                    out=y_sb[:, t8, :], in_=ps[:, 0:512], func=AF.Copy
                )
                nc.scalar.activation(
                    out=lnu[:, t8:t8 + 1], in_=mv_all[:, t8, 1:2],
                    func=AF.Ln, bias=eps_sb[:, :],
                )
                nc.scalar.activation(
                    out=rstd_all[:, t8:t8 + 1], in_=lnu[:, t8:t8 + 1],
                    func=AF.Exp, scale=-0.5,
                )
                tmp = work.tile([P, D], F32, tag="lntmp")
                nc.vector.scalar_tensor_tensor(
                    out=tmp[:, :],
                    in0=y_sb[:, t8, :],
                    scalar=mv_all[:, t8, 0:1],
                    in1=gm_sb[:, :],
                    op0=ALU.subtract,
                    op1=ALU.mult,
                )
                fin = work.tile([P, D], F32, tag="lnfin")
                nc.vector.scalar_tensor_tensor(
                    out=fin[:, :],
                    in0=tmp[:, :],
                    scalar=rstd_all[:, t8:t8 + 1],
                    in1=bt_sb[:, :],
                    op0=ALU.mult,
                    op1=ALU.add,
                )
                nc.sync.dma_start(out=outd[t8 * P:(t8 + 1) * P, :], in_=fin[:, :])

    nc.compile()
    return nc


def _prep_shared(Wq, Wk, Wv, Wo, bq, bk, bv, bo, gamma, beta):
    """Host-side shared prep: fp8 weights (split for DoubleRow) + biases."""
    e4 = ml_dtypes.float8_e4m3

    def wsplit(w):  # [dout, din] -> [p, a, k, dout] fp8, din = a*256+k*128+p
        return np.ascontiguousarray(
            w.T.reshape(2, 2, 128, 512).transpose(2, 0, 1, 3)
        ).astype(e4)

    wq8 = wsplit(Wq)
    wk8 = wsplit(Wk)
    wv8 = wsplit(Wv)

    # out-proj: ctxT row (p, a, k) holds dv = (a*4 + k*2 + p//64)*64 + p%64
    pv, av, kv = np.meshgrid(
        np.arange(128), np.arange(2), np.arange(2), indexing="ij"
    )
    dvmap = (av * 4 + kv * 2 + pv // 64) * 64 + pv % 64
    wo8 = np.ascontiguousarray(Wo.T[dvmap]).astype(e4)

    bq_p = np.ascontiguousarray(bq.reshape(4, 128).T, dtype=np.float32)
    bk_p = np.ascontiguousarray(bk.reshape(4, 128).T, dtype=np.float32)
    bo_eff = (bo + Wo @ bv).astype(np.float32)
    gm_n = np.ascontiguousarray(
        np.broadcast_to(gamma[None, :], (P, D)), dtype=np.float32)
    bt_n = np.ascontiguousarray(
        np.broadcast_to(beta[None, :], (P, D)), dtype=np.float32)
    id128 = np.eye(P).astype(ml_dtypes.bfloat16)
    return wq8, wk8, wv8, wo8, bq_p, bk_p, bo_eff, gm_n, bt_n, id128


def prepare_in_maps(x, Wq, bq, Wk, bk, Wv, bv, Wo, bo, gamma, beta):
    e4 = ml_dtypes.float8_e4m3
    x = np.asarray(x, np.float32)
    args = [np.asarray(v, np.float32)
            for v in (Wq, Wk, Wv, Wo, bq, bk, bv, bo, gamma, beta)]
    (wq8, wk8, wv8, wo8, bq_p, bk_p, bo_eff, gm_n, bt_n, id128) = _prep_shared(
        args[0], args[1], args[2], args[3], args[4], args[5], args[6],
        args[7], args[8], args[9])

    in_maps = []
    for c in range(NCORES):
        b = c // 2
        par = c % 2
        xt = x[b].T  # [D, S]
        xt8 = np.ascontiguousarray(
            xt.reshape(2, 2, 128, S).transpose(2, 0, 1, 3)).astype(e4)
        xtq8 = np.ascontiguousarray(
            xt[:, par * TQ:(par + 1) * TQ]
            .reshape(2, 2, 128, TQ).transpose(2, 0, 1, 3)).astype(e4)
        xres = (x[b, par * TQ:(par + 1) * TQ] + bo_eff[None, :]).astype(
            np.float32)
        xresb = np.ascontiguousarray(
            xres.reshape(8, 128, D).transpose(1, 0, 2)).astype(
            ml_dtypes.bfloat16)
        in_maps.append({
            "xt8": xt8, "xtq8": xtq8,
            "wq8": wq8, "wk8": wk8, "wv8": wv8, "wo8": wo8,
            "bqp": bq_p, "bkp": bk_p,
            "xres": xresb, "id128": id128,
            "gamma": gm_n, "beta": bt_n,
        })
    return in_maps


def _get_nc():
    global _NC_CACHE
    if _NC_CACHE is None:
        _NC_CACHE = _build()
    return _NC_CACHE


def kernel(x, Wq, bq, Wk, bk, Wv, bv, Wo, bo, gamma, beta):
    global LAST_RESULTS
    in_maps = prepare_in_maps(x, Wq, bq, Wk, bk, Wv, bv, Wo, bo, gamma, beta)
    nc = _get_nc()
    res = run_bass_kernel_spmd(
        nc, in_maps, core_ids=list(range(NCORES)), trace=TRACE)
    LAST_RESULTS = res

    outf = np.empty((B, S, D), np.float32)
    for c in range(NCORES):
        b = c // 2
        par = c % 2
        outf[b, par * TQ:(par + 1) * TQ, :] = res.results[c]["out"]
    return outf
